# revision 10
# baseline (speedup 1.0000x reference)
"""MLA (DeepseekV3-style) attention kernel for 8 Trainium2 NeuronCores.

Contract: kernel(**inputs) takes the FULL unsharded numpy inputs
(q_nope [2,2048,16,128], q_pe [2,2048,16,64], k_nope [2,2048,16,128],
k_pe [2,2048,1,64], v [2,2048,16,128], all float32) and returns
(o [4096,16,128] f32, lse [2,2048,16] f32) exactly like the reference.

Strategy: shard the 32 (b, h) pairs over 8 cores, 4 pairs/core, b-major
(each core sees a single batch index, so the shared k_pe rope is computed
once per core).  The host only does layout work (transposes / packing /
constant yarn tables); all FLOPs (rope, matmuls, softmax) run on device.

Device kernel (per core, per pair):
  S^T[n, m] = (k_nope^T q_nope + k_rope^T q_rope)          # 2 accum matmuls
  P^T = exp(S^T * sm_scale)   (ACT), causal mask via gpsimd
  O^T[dv, m] += V[n_block]^T-stationary matmul over n blocks  (PSUM accum)
  den[m]     += ones-matmul over n blocks                      (PSUM accum)
  lse = ln(den) * log2(e);  o = O^T * exp(-ln(den))  (normalize on evac)
Host transposes O^T back and assembles the full outputs.
"""

import functools
import math
import sys

import numpy as np

for _p in ("/opt/trn_rl_repo", "/root/.axon_site/_ro/trn_rl_repo"):
    if _p not in sys.path:
        sys.path.insert(0, _p)

import concourse.bacc as bacc
import concourse.mybir as mybir
import concourse.tile as tile
from concourse.bass_utils import run_bass_kernel_spmd

F32 = mybir.dt.float32
F32R = mybir.dt.float32r
BF16 = mybir.dt.bfloat16

B, S, H, DN, DR, DV = 2, 2048, 16, 128, 64, 128
NCORES = 8
NPAIRS = (B * H) // NCORES  # 4 (b,h) pairs per core
MC = 512  # m-chunk width = one fp32 PSUM bank
SM_SCALE = (DN + DR) ** -0.5
LOG2E = 1.0 / math.log(2.0)

# ---- yarn rope tables (constants; mirrors reference.py) --------------------
BASE = 10000.0
SCALING_FACTOR = 1.0
ORIG_MAX_POS = 4096
BETA_FAST, BETA_SLOW = 32, 1
MSCALE = 0.707
MSCALE_ALL_DIM = 0.707


def _yarn_cos_sin(seq_len: int, dim: int):
    def corr_dim(num_rot):
        return (
            dim
            * math.log(ORIG_MAX_POS / (num_rot * 2 * math.pi))
            / (2 * math.log(BASE))
        )

    low = max(math.floor(corr_dim(BETA_FAST)), 0)
    high = min(math.ceil(corr_dim(BETA_SLOW)), dim - 1)
    hi = high + 0.001 if low == high else high
    ramp = np.clip((np.arange(dim // 2, dtype=np.float32) - low) / (hi - low), 0.0, 1.0)
    inv_freq_mask = 1.0 - ramp
    freq_extra = 1.0 / BASE ** (np.arange(0, dim, 2, dtype=np.float32) / dim)
    freq_inter = freq_extra / SCALING_FACTOR
    inv_freq = freq_inter * (1.0 - inv_freq_mask) + freq_extra * inv_freq_mask
    t = np.arange(seq_len, dtype=np.float32)
    freqs = np.outer(t, inv_freq)

    def get_mscale(s, m):
        return 1.0 if s <= 1 else 0.1 * m * math.log(s) + 1.0

    _m = get_mscale(SCALING_FACTOR, MSCALE) / get_mscale(SCALING_FACTOR, MSCALE_ALL_DIM)
    emb = np.concatenate([freqs, freqs], axis=-1)  # [S, dim]
    return (np.cos(emb) * _m).astype(np.float32), (np.sin(emb) * _m).astype(np.float32)


# ---- device kernel builder -------------------------------------------------
def build_nc(npairs=NPAIRS, s=S, mm_dt=F32R, debug=False):
    """Build the single-core SPMD Bass program (same program on all cores)."""
    ntt = s // 128  # number of 128-wide n (kv) tiles
    nmc = s // MC  # number of 512-wide m (query) chunks
    ngroups = (npairs + 1) // 2
    mdt = mm_dt  # dtype of every matmul operand, typed end-to-end for walrus
    nc = bacc.Bacc("TRN2", target_bir_lowering=False, debug=debug)

    q_nope_t = nc.dram_tensor("q_nope_t", [npairs, 128, s], mdt, kind="ExternalInput")
    k_nope_t = nc.dram_tensor("k_nope_t", [npairs, 128, s], mdt, kind="ExternalInput")
    q_pe_nat = nc.dram_tensor("q_pe_nat", [ngroups, 128, s], mdt, kind="ExternalInput")
    q_pe_rot = nc.dram_tensor("q_pe_rot", [ngroups, 128, s], mdt, kind="ExternalInput")
    k_pe_nat = nc.dram_tensor("k_pe_nat", [64, s], mdt, kind="ExternalInput")
    k_pe_rot = nc.dram_tensor("k_pe_rot", [64, s], mdt, kind="ExternalInput")
    cs = nc.dram_tensor("cs", [2, 128, s], mdt, kind="ExternalInput")
    v_t = nc.dram_tensor("v_t", [npairs, 128, ntt, 128], mdt, kind="ExternalInput")
    o_t = nc.dram_tensor("o_t", [npairs, 128, s], F32, kind="ExternalOutput")
    lse = nc.dram_tensor("lse", [npairs, s], F32, kind="ExternalOutput")

    Exp = mybir.ActivationFunctionType.Exp
    Ln = mybir.ActivationFunctionType.Ln
    Copy = mybir.ActivationFunctionType.Copy

    with tile.TileContext(nc) as tc:
        with (
            tc.tile_pool(name="const", bufs=1) as constp,
            tc.tile_pool(name="ropein", bufs=1) as ropein,
            tc.tile_pool(name="ropeout", bufs=1) as ropeout,
            tc.tile_pool(name="pairin", bufs=2) as pairin,
            tc.tile_pool(name="pmat", bufs=6) as pmat,
            tc.tile_pool(name="osb", bufs=2) as osbp,
            tc.tile_pool(name="rows", bufs=2) as rows,
            tc.tile_pool(name="ps_s", bufs=2, space="PSUM") as ps_s,
            tc.tile_pool(name="ps_o", bufs=2, space="PSUM") as ps_o,
            tc.tile_pool(name="ps_d", bufs=2, space="PSUM") as ps_d,
        ):
            cos2 = constp.tile([128, s], mdt)
            nc.sync.dma_start(cos2, cs[0])
            sin2 = constp.tile([128, s], mdt)
            nc.sync.dma_start(sin2, cs[1])
            ones = constp.tile([128, 128], mdt)
            zeros = constp.tile([128, 384], mdt)
            if mdt == F32:
                nc.vector.memset(ones, 1.0)
                nc.vector.memset(zeros, 0.0)
            else:
                # Memset can't write f32r; memset f32 scratch, round via copy
                ones_f32 = constp.tile([128, 384], F32)
                nc.vector.memset(ones_f32, 1.0)
                nc.vector.tensor_copy(ones, ones_f32[:, 0:128])
                nc.vector.memset(ones_f32, 0.0)
                nc.vector.tensor_copy(zeros, ones_f32)

            # k_pe rope — shared by every pair on this core (same b)
            k_nat = ropein.tile([64, s], mdt, tag="knat")
            nc.sync.dma_start(k_nat, k_pe_nat[:])
            k_rot = ropein.tile([64, s], mdt, tag="krot")
            nc.sync.dma_start(k_rot, k_pe_rot[:])
            kp = ropeout.tile([64, s], mdt, tag="kp")
            nc.vector.tensor_mul(k_rot, k_rot, sin2[0:64, :])  # in-place
            nc.vector.tensor_mul(kp, k_nat, cos2[0:64, :])
            nc.vector.tensor_add(kp, kp, k_rot)

            # q_pe rope — two pairs packed per [128, s] group
            qps = []
            for g in range(ngroups):
                qn = ropein.tile([128, s], mdt, tag="qnat")
                nc.sync.dma_start(qn, q_pe_nat[g])
                qr = ropein.tile([128, s], mdt, tag="qrot")
                nc.sync.dma_start(qr, q_pe_rot[g])
                qp = ropeout.tile([128, s], mdt, tag=f"qp{g}")
                nc.vector.tensor_mul(qr, qr, sin2)  # in-place
                nc.vector.tensor_mul(qp, qn, cos2)
                nc.vector.tensor_add(qp, qp, qr)
                # matmul requires lhsT/rhs at the same base partition; move the
                # packed upper half (odd pair) down to base 0 via SBUF-SBUF DMA
                qp_hi = ropeout.tile([64, s], mdt, tag=f"qp_hi{g}")
                nc.sync.dma_start(qp_hi, qp[64:128, :])
                qps.append((qp, qp_hi))

            for p in range(npairs):
                g, half = divmod(p, 2)
                qp_pair = qps[g][0][0:64, :] if half == 0 else qps[g][1][:, :]
                qn_t = pairin.tile([128, s], mdt, tag="qn")
                nc.sync.dma_start(qn_t, q_nope_t[p])
                kn_t = pairin.tile([128, s], mdt, tag="kn")
                nc.sync.dma_start(kn_t, k_nope_t[p])
                v_sb = pairin.tile([128, ntt, 128], mdt, tag="v")
                nc.sync.dma_start(v_sb, v_t[p])
                lse_sb = rows.tile([1, s], F32, tag="lse")

                for mc in range(nmc):
                    msl = slice(mc * MC, (mc + 1) * MC)
                    nlim = (mc + 1) * (MC // 128)
                    psum_o = ps_o.tile([128, MC], F32)
                    psum_d = ps_d.tile([128, MC], F32)
                    pending = None  # software-pipeline PV/den one step behind

                    def emit_pv(p_t, nt, nlim=nlim, psum_o=psum_o, psum_d=psum_d):
                        nc.tensor.matmul(
                            psum_o,
                            v_sb[:, nt, :],
                            p_t,
                            start=(nt == 0),
                            stop=(nt == nlim - 1),
                        )
                        nc.tensor.matmul(
                            psum_d,
                            ones,
                            p_t,
                            start=(nt == 0),
                            stop=(nt == nlim - 1),
                        )

                    for nt in range(nlim):
                        nsl = slice(nt * 128, (nt + 1) * 128)
                        psum_s = ps_s.tile([128, MC], F32)
                        nc.tensor.matmul(
                            psum_s,
                            kn_t[:, nsl],
                            qn_t[:, msl],
                            start=True,
                            stop=False,
                        )
                        nc.tensor.matmul(
                            psum_s,
                            kp[:, nsl],
                            qp_pair[:, msl],
                            start=False,
                            stop=True,
                        )
                        p_t = pmat.tile([128, MC], mdt, tag="pt")
                        diag_k = nt - (nlim - MC // 128)  # >=0 inside diag band
                        off = 128 * diag_k if diag_k > 0 else 0
                        if off > 0:
                            nc.gpsimd.tensor_copy(p_t[:, 0:off], zeros[:, 0:off])
                        nc.scalar.activation(
                            p_t[:, off:], psum_s[:, off:], Exp, scale=SM_SCALE
                        )
                        if diag_k >= 0:
                            # keep m_local >= n_local within the diagonal block
                            nc.gpsimd.affine_select(
                                out=p_t[:, off : off + 128],
                                in_=p_t[:, off : off + 128],
                                compare_op=mybir.AluOpType.is_ge,
                                fill=0.0,
                                base=0,
                                pattern=[[1, 128]],
                                channel_multiplier=-1,
                            )
                        if pending is not None:
                            emit_pv(*pending)
                        pending = (p_t, nt)
                    emit_pv(*pending)

                    # epilogue: lse + normalization
                    lnden = rows.tile([1, MC], F32, tag="lnden")
                    nc.scalar.activation(lnden, psum_d[0:1, :], Ln)
                    nc.scalar.activation(lse_sb[:, msl], lnden, Copy, scale=LOG2E)
                    recip = rows.tile([1, MC], F32, tag="recip")
                    nc.scalar.activation(recip, lnden, Exp, scale=-1.0)
                    rb = osbp.tile([128, MC], F32, tag="rb")
                    nc.gpsimd.partition_broadcast(rb, recip)
                    o_sb = osbp.tile([128, MC], F32, tag="osb")
                    nc.vector.tensor_mul(o_sb, psum_o, rb)
                    nc.sync.dma_start(o_t[p][:, msl], o_sb)
                nc.sync.dma_start(lse[p : p + 1, :], lse_sb)

    nc.compile()
    return nc


# ---- host-side input/output marshalling ------------------------------------
def _pair_index(core, i):
    idx = core * NPAIRS + i  # b-major: cores 0-3 -> b=0, cores 4-7 -> b=1
    return idx // H, idx % H


def make_in_maps(q_nope, q_pe, k_nope, k_pe, v, npairs=NPAIRS, s=S, ncores=NCORES):
    """Slice + lay out the full inputs into per-core input dicts."""
    cos, sin = _yarn_cos_sin(s, DR)
    # device computes qp[r] = nat[r]*cos[r] + nat[(r+32)%64]*sin_eff[r]
    sin_eff = sin.copy()
    sin_eff[:, : DR // 2] *= -1.0
    cosT = np.ascontiguousarray(cos.T)  # [64, S]
    sinT = np.ascontiguousarray(sin_eff.T)
    cs = np.stack(
        [np.concatenate([cosT, cosT], 0), np.concatenate([sinT, sinT], 0)]
    )  # [2, 128, S]

    ntt = s // 128
    ngroups = (npairs + 1) // 2
    in_maps = []
    for c in range(ncores):
        pairs = [_pair_index(c, i) for i in range(npairs)]
        b0 = pairs[0][0]
        assert all(b == b0 for b, _ in pairs), "core must own a single batch"
        qnt = np.stack([np.ascontiguousarray(q_nope[b, :, h, :].T) for b, h in pairs])
        knt = np.stack([np.ascontiguousarray(k_nope[b, :, h, :].T) for b, h in pairs])
        vt = np.stack(
            [
                np.ascontiguousarray(
                    v[b, :, h, :].reshape(ntt, 128, DV).transpose(1, 0, 2)
                )
                for b, h in pairs
            ]
        )
        q_nat_groups, q_rot_groups = [], []
        for gi in range(ngroups):
            mats_nat, mats_rot = [], []
            for i in (2 * gi, min(2 * gi + 1, npairs - 1)):
                b, h = pairs[i]
                nat = np.ascontiguousarray(q_pe[b, :, h, :].T)  # [64, S]
                mats_nat.append(nat)
                mats_rot.append(np.concatenate([nat[DR // 2 :], nat[: DR // 2]], 0))
            q_nat_groups.append(np.concatenate(mats_nat, 0))
            q_rot_groups.append(np.concatenate(mats_rot, 0))
        k_nat = np.ascontiguousarray(k_pe[b0, :, 0, :].T)  # [64, S]
        k_rot = np.concatenate([k_nat[DR // 2 :], k_nat[: DR // 2]], 0)
        in_maps.append(
            {
                "q_nope_t": qnt.astype(np.float32),
                "k_nope_t": knt.astype(np.float32),
                "q_pe_nat": np.stack(q_nat_groups).astype(np.float32),
                "q_pe_rot": np.stack(q_rot_groups).astype(np.float32),
                "k_pe_nat": k_nat.astype(np.float32),
                "k_pe_rot": k_rot.astype(np.float32),
                "cs": cs.astype(np.float32),
                "v_t": vt.astype(np.float32),
            }
        )
    return in_maps


def assemble_outputs(results, npairs=NPAIRS, s=S, ncores=NCORES):
    o = np.empty((B, s, H, DV), np.float32)
    lse_full = np.empty((B, s, H), np.float32)
    for c in range(ncores):
        r = results[c]
        for i in range(npairs):
            b, h = _pair_index(c, i)
            o[b, :, h, :] = r["o_t"][i].T
            lse_full[b, :, h] = r["lse"][i]
    return o.reshape(B * s, H, DV), lse_full


@functools.lru_cache(maxsize=1)
def _get_nc():
    return build_nc()


def kernel(q_nope, q_pe, k_nope, k_pe, v, _trace=False):
    q_nope = np.asarray(q_nope, np.float32)
    q_pe = np.asarray(q_pe, np.float32)
    k_nope = np.asarray(k_nope, np.float32)
    k_pe = np.asarray(k_pe, np.float32)
    v = np.asarray(v, np.float32)
    in_maps = make_in_maps(q_nope, q_pe, k_nope, k_pe, v)
    nc = _get_nc()
    res = run_bass_kernel_spmd(nc, in_maps, core_ids=list(range(NCORES)), trace=_trace)
    out = assemble_outputs(res.results)
    if _trace:
        return out, res
    return out


# revision 15
# speedup vs baseline: 1.2700x; 1.2700x over previous
"""MLA (DeepseekV3-style) attention kernel for 8 Trainium2 NeuronCores.

Contract: kernel(**inputs) takes the FULL unsharded numpy inputs
(q_nope [2,2048,16,128], q_pe [2,2048,16,64], k_nope [2,2048,16,128],
k_pe [2,2048,1,64], v [2,2048,16,128], all float32) and returns
(o [4096,16,128] f32, lse [2,2048,16] f32) exactly like the reference.

Strategy: shard the 32 (b, h) pairs over 8 cores, 4 pairs/core, b-major
(each core sees a single batch index, so the shared k_pe rope is computed
once per core).  The host only does layout work (transposes / packing /
constant yarn tables); all FLOPs (rope, matmuls, softmax) run on device.

Device kernel (per core, per pair):
  S^T[n, m] = (k_nope^T q_nope + k_rope^T q_rope)          # 2 accum matmuls
  P^T = exp(S^T * sm_scale)   (ACT), causal mask via gpsimd
  O^T[dv, m] += V[n_block]^T-stationary matmul over n blocks  (PSUM accum)
  den[m]     += ones-matmul over n blocks                      (PSUM accum)
  lse = ln(den) * log2(e);  o = O^T * exp(-ln(den))  (normalize on evac)
Host transposes O^T back and assembles the full outputs.
"""

import functools
import math
import sys

import numpy as np

for _p in ("/opt/trn_rl_repo", "/root/.axon_site/_ro/trn_rl_repo"):
    if _p not in sys.path:
        sys.path.insert(0, _p)

import concourse.bacc as bacc
import concourse.mybir as mybir
import concourse.tile as tile
from concourse.bass_utils import run_bass_kernel_spmd

F32 = mybir.dt.float32
F32R = mybir.dt.float32r
BF16 = mybir.dt.bfloat16

B, S, H, DN, DR, DV = 2, 2048, 16, 128, 64, 128
NCORES = 8
NPAIRS = (B * H) // NCORES  # 4 (b,h) pairs per core
MC = 512  # m-chunk width = one fp32 PSUM bank
SM_SCALE = (DN + DR) ** -0.5
LOG2E = 1.0 / math.log(2.0)

# ---- yarn rope tables (constants; mirrors reference.py) --------------------
BASE = 10000.0
SCALING_FACTOR = 1.0
ORIG_MAX_POS = 4096
BETA_FAST, BETA_SLOW = 32, 1
MSCALE = 0.707
MSCALE_ALL_DIM = 0.707


def _yarn_cos_sin(seq_len: int, dim: int):
    def corr_dim(num_rot):
        return (
            dim
            * math.log(ORIG_MAX_POS / (num_rot * 2 * math.pi))
            / (2 * math.log(BASE))
        )

    low = max(math.floor(corr_dim(BETA_FAST)), 0)
    high = min(math.ceil(corr_dim(BETA_SLOW)), dim - 1)
    hi = high + 0.001 if low == high else high
    ramp = np.clip((np.arange(dim // 2, dtype=np.float32) - low) / (hi - low), 0.0, 1.0)
    inv_freq_mask = 1.0 - ramp
    freq_extra = 1.0 / BASE ** (np.arange(0, dim, 2, dtype=np.float32) / dim)
    freq_inter = freq_extra / SCALING_FACTOR
    inv_freq = freq_inter * (1.0 - inv_freq_mask) + freq_extra * inv_freq_mask
    t = np.arange(seq_len, dtype=np.float32)
    freqs = np.outer(t, inv_freq)

    def get_mscale(s, m):
        return 1.0 if s <= 1 else 0.1 * m * math.log(s) + 1.0

    _m = get_mscale(SCALING_FACTOR, MSCALE) / get_mscale(SCALING_FACTOR, MSCALE_ALL_DIM)
    emb = np.concatenate([freqs, freqs], axis=-1)  # [S, dim]
    return (np.cos(emb) * _m).astype(np.float32), (np.sin(emb) * _m).astype(np.float32)


# ---- device kernel builder -------------------------------------------------
def build_nc(npairs=NPAIRS, s=S, mm_dt=F32R, debug=False):
    """Build the single-core SPMD Bass program (same program on all cores)."""
    ntt = s // 128  # number of 128-wide n (kv) tiles
    nmc = s // MC  # number of 512-wide m (query) chunks
    ngroups = (npairs + 1) // 2
    mdt = mm_dt  # dtype of every matmul operand, typed end-to-end for walrus
    nc = bacc.Bacc("TRN2", target_bir_lowering=False, debug=debug)

    q_nope_t = nc.dram_tensor("q_nope_t", [npairs, 128, s], mdt, kind="ExternalInput")
    k_nope_t = nc.dram_tensor("k_nope_t", [npairs, 128, s], mdt, kind="ExternalInput")
    q_pe_nat = nc.dram_tensor("q_pe_nat", [ngroups, 128, s], mdt, kind="ExternalInput")
    q_pe_rot = nc.dram_tensor("q_pe_rot", [ngroups, 128, s], mdt, kind="ExternalInput")
    k_pe_nat = nc.dram_tensor("k_pe_nat", [64, s], mdt, kind="ExternalInput")
    k_pe_rot = nc.dram_tensor("k_pe_rot", [64, s], mdt, kind="ExternalInput")
    cs = nc.dram_tensor("cs", [2, 128, s], mdt, kind="ExternalInput")
    v_t = nc.dram_tensor("v_t", [npairs, 128, ntt, 128], mdt, kind="ExternalInput")
    o_t = nc.dram_tensor("o_t", [npairs, 128, s], F32, kind="ExternalOutput")
    lse = nc.dram_tensor("lse", [npairs, s], F32, kind="ExternalOutput")

    Exp = mybir.ActivationFunctionType.Exp
    Ln = mybir.ActivationFunctionType.Ln
    Copy = mybir.ActivationFunctionType.Copy

    with tile.TileContext(nc) as tc:
        with (
            tc.tile_pool(name="const", bufs=1) as constp,
            tc.tile_pool(name="ropein", bufs=1) as ropein,
            tc.tile_pool(name="ropeout", bufs=1) as ropeout,
            tc.tile_pool(name="pairin", bufs=2) as pairin,
            tc.tile_pool(name="pmat", bufs=4) as pmat,
            tc.tile_pool(name="oun", bufs=2) as ounp,
            tc.tile_pool(name="rbp", bufs=1) as rbp,
            tc.tile_pool(name="rows", bufs=1) as rows,
            tc.tile_pool(name="ps_s", bufs=2, space="PSUM") as ps_s,
            tc.tile_pool(name="ps_o", bufs=2, space="PSUM") as ps_o,
            tc.tile_pool(name="ps_d", bufs=2, space="PSUM") as ps_d,
        ):
            cos2 = constp.tile([128, s], mdt)
            nc.sync.dma_start(cos2, cs[0])
            sin2 = constp.tile([128, s], mdt)
            nc.sync.dma_start(sin2, cs[1])
            ones = constp.tile([128, 128], mdt)
            if mdt == F32:
                nc.vector.memset(ones, 1.0)
            else:
                ones_f32 = constp.tile([128, 128], F32)
                nc.vector.memset(ones_f32, 1.0)
                nc.vector.tensor_copy(ones, ones_f32)  # rounds to mm dtype

            # k_pe rope - shared by every pair on this core (same b)
            k_nat = ropein.tile([64, s], mdt, tag="knat")
            nc.sync.dma_start(k_nat, k_pe_nat[:])
            k_rot = ropein.tile([64, s], mdt, tag="krot")
            nc.sync.dma_start(k_rot, k_pe_rot[:])
            kp = ropeout.tile([64, s], mdt, tag="kp")
            nc.vector.tensor_mul(k_rot, k_rot, sin2[0:64, :])  # in-place
            nc.vector.tensor_mul(kp, k_nat, cos2[0:64, :])
            nc.vector.tensor_add(kp, kp, k_rot)

            # q_pe rope - two pairs packed per [128, s] group
            qps = []
            for g in range(ngroups):
                qn = ropein.tile([128, s], mdt, tag="qnat")
                nc.sync.dma_start(qn, q_pe_nat[g])
                qr = ropein.tile([128, s], mdt, tag="qrot")
                nc.sync.dma_start(qr, q_pe_rot[g])
                qp = ropeout.tile([128, s], mdt, tag=f"qp{g}")
                nc.vector.tensor_mul(qr, qr, sin2)  # in-place
                nc.vector.tensor_mul(qp, qn, cos2)
                nc.vector.tensor_add(qp, qp, qr)
                # matmul requires lhsT/rhs at the same base partition; move the
                # packed upper half (odd pair) down to base 0 via SBUF-SBUF DMA
                qp_hi = ropeout.tile([64, s], mdt, tag=f"qp_hi{g}")
                nc.sync.dma_start(qp_hi, qp[64:128, :])
                qps.append((qp, qp_hi))

            NTB = MC // 128  # n-tiles per m-chunk (4)
            for p in range(npairs):
                g, half = divmod(p, 2)
                qp_pair = qps[g][0][0:64, :] if half == 0 else qps[g][1][:, :]
                qn_t = pairin.tile([128, s], mdt, tag="qn")
                nc.sync.dma_start(qn_t, q_nope_t[p])
                kn_t = pairin.tile([128, s], mdt, tag="kn")
                nc.sync.dma_start(kn_t, k_nope_t[p])
                v_sb = pairin.tile([128, ntt, 128], mdt, tag="v")
                nc.sync.dma_start(v_sb, v_t[p])
                o_un = ounp.tile([128, s], F32, tag="oun")
                den_sb = rows.tile([1, s], F32, tag="den")

                for mc in range(nmc):
                    msl = slice(mc * MC, (mc + 1) * MC)
                    nlim = (mc + 1) * NTB
                    psum_o = ps_o.tile([128, MC], F32)
                    psum_d = ps_d.tile([128, MC], F32)
                    pending = None  # software-pipeline PV/den one step behind

                    def emit_pv(p_t, pvs, nts, nlim=nlim, psum_o=psum_o, psum_d=psum_d):
                        for h, (vs, nt) in enumerate(zip(pvs, nts)):
                            hsl = slice(h * 512 + vs, (h + 1) * 512)
                            nc.tensor.matmul(
                                psum_o[:, vs:],
                                v_sb[:, nt, :],
                                p_t[:, hsl],
                                start=(nt == 0),
                                stop=(nt == nlim - 1),
                            )
                            nc.tensor.matmul(
                                psum_d[:, vs:],
                                ones,
                                p_t[:, hsl],
                                start=(nt == 0),
                                stop=(nt == nlim - 1),
                            )

                    for t in range(nlim // 2):
                        nts = (2 * t, 2 * t + 1)
                        psum_s = ps_s.tile([128, 2, MC], F32)
                        p_t = pmat.tile([128, 2, MC], mdt, tag="pt")
                        offs, pvs = [], []
                        for h, nt in enumerate(nts):
                            diag_k = nt - (nlim - NTB)  # >=0 inside diag band
                            off = 128 * diag_k if diag_k > 0 else 0
                            vs = min(off, 256)  # keep f32r moving width >= 256
                            offs.append(off)
                            pvs.append(vs)
                            nsl = slice(nt * 128, (nt + 1) * 128)
                            qsl = slice(mc * MC + vs, (mc + 1) * MC)
                            nc.tensor.matmul(
                                psum_s[:, h, vs:],
                                kn_t[:, nsl],
                                qn_t[:, qsl],
                                start=True,
                                stop=False,
                            )
                            nc.tensor.matmul(
                                psum_s[:, h, vs:],
                                kp[:, nsl],
                                qp_pair[:, qsl],
                                start=False,
                                stop=True,
                            )
                        # one wide exp over both halves when fully valid;
                        # split per half when a diagonal offset leaves the
                        # head of a half unwritten in PSUM
                        if pvs[0] == 0 and pvs[1] == 0:
                            nc.scalar.activation(
                                p_t.rearrange("p a b -> p (a b)"),
                                psum_s.rearrange("p a b -> p (a b)"),
                                Exp,
                                scale=SM_SCALE,
                            )
                        else:
                            for h in range(2):
                                nc.scalar.activation(
                                    p_t[:, h, pvs[h] :],
                                    psum_s[:, h, pvs[h] :],
                                    Exp,
                                    scale=SM_SCALE,
                                )
                        for h, nt in enumerate(nts):
                            diag_k = nt - (nlim - NTB)
                            if diag_k >= 0:
                                off, vs = offs[h], pvs[h]
                                w = off + 128 - vs
                                # keep cols where m_local >= n_local:
                                # iota = (y + vs - off) - x >= 0
                                nc.gpsimd.affine_select(
                                    out=p_t[:, h, vs : off + 128],
                                    in_=p_t[:, h, vs : off + 128],
                                    compare_op=mybir.AluOpType.is_ge,
                                    fill=0.0,
                                    base=vs - off,
                                    pattern=[[1, w]],
                                    channel_multiplier=-1,
                                )
                        if pending is not None:
                            emit_pv(*pending)
                        pending = (p_t.rearrange("p a b -> p (a b)"), pvs, nts)
                    emit_pv(*pending)

                    # evacuate den row (all psum_d rows are identical; read the
                    # one matching this chunk's den_sb partition to stay aligned)
                    nc.vector.tensor_copy(
                        den_sb[:, msl], psum_d[0:1, :]
                    )
                    # evacuate unnormalized O^T
                    nc.vector.tensor_copy(o_un[:, msl], psum_o)

                # deferred per-pair epilogue: one ACT table context
                nc.scalar.activation(den_sb, den_sb, Ln)
                lse_sb = rows.tile([1, s], F32, tag="lse")
                nc.scalar.activation(lse_sb, den_sb, Copy, scale=LOG2E)
                nc.scalar.activation(den_sb, den_sb, Exp, scale=-1.0)  # 1/den
                rb = rbp.tile([128, s], F32, tag="rb")
                nc.gpsimd.partition_broadcast(rb, den_sb)
                nc.vector.tensor_mul(o_un, o_un, rb)
                nc.sync.dma_start(o_t[p], o_un)
                nc.sync.dma_start(lse[p : p + 1, :], lse_sb)

    nc.compile()
    return nc


# ---- host-side input/output marshalling ------------------------------------
def _pair_index(core, i):
    idx = core * NPAIRS + i  # b-major: cores 0-3 -> b=0, cores 4-7 -> b=1
    return idx // H, idx % H


def make_in_maps(q_nope, q_pe, k_nope, k_pe, v, npairs=NPAIRS, s=S, ncores=NCORES):
    """Slice + lay out the full inputs into per-core input dicts."""
    cos, sin = _yarn_cos_sin(s, DR)
    # device computes qp[r] = nat[r]*cos[r] + nat[(r+32)%64]*sin_eff[r]
    sin_eff = sin.copy()
    sin_eff[:, : DR // 2] *= -1.0
    cosT = np.ascontiguousarray(cos.T)  # [64, S]
    sinT = np.ascontiguousarray(sin_eff.T)
    cs = np.stack(
        [np.concatenate([cosT, cosT], 0), np.concatenate([sinT, sinT], 0)]
    )  # [2, 128, S]

    ntt = s // 128
    ngroups = (npairs + 1) // 2
    in_maps = []
    for c in range(ncores):
        pairs = [_pair_index(c, i) for i in range(npairs)]
        b0 = pairs[0][0]
        assert all(b == b0 for b, _ in pairs), "core must own a single batch"
        qnt = np.stack([np.ascontiguousarray(q_nope[b, :, h, :].T) for b, h in pairs])
        knt = np.stack([np.ascontiguousarray(k_nope[b, :, h, :].T) for b, h in pairs])
        vt = np.stack(
            [
                np.ascontiguousarray(
                    v[b, :, h, :].reshape(ntt, 128, DV).transpose(1, 0, 2)
                )
                for b, h in pairs
            ]
        )
        q_nat_groups, q_rot_groups = [], []
        for gi in range(ngroups):
            mats_nat, mats_rot = [], []
            for i in (2 * gi, min(2 * gi + 1, npairs - 1)):
                b, h = pairs[i]
                nat = np.ascontiguousarray(q_pe[b, :, h, :].T)  # [64, S]
                mats_nat.append(nat)
                mats_rot.append(np.concatenate([nat[DR // 2 :], nat[: DR // 2]], 0))
            q_nat_groups.append(np.concatenate(mats_nat, 0))
            q_rot_groups.append(np.concatenate(mats_rot, 0))
        k_nat = np.ascontiguousarray(k_pe[b0, :, 0, :].T)  # [64, S]
        k_rot = np.concatenate([k_nat[DR // 2 :], k_nat[: DR // 2]], 0)
        in_maps.append(
            {
                "q_nope_t": qnt.astype(np.float32),
                "k_nope_t": knt.astype(np.float32),
                "q_pe_nat": np.stack(q_nat_groups).astype(np.float32),
                "q_pe_rot": np.stack(q_rot_groups).astype(np.float32),
                "k_pe_nat": k_nat.astype(np.float32),
                "k_pe_rot": k_rot.astype(np.float32),
                "cs": cs.astype(np.float32),
                "v_t": vt.astype(np.float32),
            }
        )
    return in_maps


def assemble_outputs(results, npairs=NPAIRS, s=S, ncores=NCORES):
    o = np.empty((B, s, H, DV), np.float32)
    lse_full = np.empty((B, s, H), np.float32)
    for c in range(ncores):
        r = results[c]
        for i in range(npairs):
            b, h = _pair_index(c, i)
            o[b, :, h, :] = r["o_t"][i].T
            lse_full[b, :, h] = r["lse"][i]
    return o.reshape(B * s, H, DV), lse_full


@functools.lru_cache(maxsize=1)
def _get_nc():
    return build_nc()


def kernel(q_nope, q_pe, k_nope, k_pe, v, _trace=False):
    q_nope = np.asarray(q_nope, np.float32)
    q_pe = np.asarray(q_pe, np.float32)
    k_nope = np.asarray(k_nope, np.float32)
    k_pe = np.asarray(k_pe, np.float32)
    v = np.asarray(v, np.float32)
    in_maps = make_in_maps(q_nope, q_pe, k_nope, k_pe, v)
    nc = _get_nc()
    res = run_bass_kernel_spmd(nc, in_maps, core_ids=list(range(NCORES)), trace=_trace)
    out = assemble_outputs(res.results)
    if _trace:
        return out, res
    return out


# revision 17
# speedup vs baseline: 1.5872x; 1.2498x over previous
"""MLA (DeepseekV3-style) attention kernel for 8 Trainium2 NeuronCores.

Contract: kernel(**inputs) takes the FULL unsharded numpy inputs
(q_nope [2,2048,16,128], q_pe [2,2048,16,64], k_nope [2,2048,16,128],
k_pe [2,2048,1,64], v [2,2048,16,128], all float32) and returns
(o [4096,16,128] f32, lse [2,2048,16] f32) exactly like the reference.

Strategy: shard the 32 (b, h) pairs over 8 cores, 4 pairs/core, b-major
(each core sees a single batch index, so the shared k_pe rope is computed
once per core).  The host only does layout work (transposes / packing /
constant yarn tables); all FLOPs (rope, matmuls, softmax) run on device.

Device kernel (per core, per pair):
  S^T[n, m] = (k_nope^T q_nope + k_rope^T q_rope)          # 2 accum matmuls
  P^T = exp(S^T * sm_scale)   (ACT), causal mask via gpsimd
  O^T[dv, m] += V[n_block]^T-stationary matmul over n blocks  (PSUM accum)
  den[m]     += ones-matmul over n blocks                      (PSUM accum)
  lse = ln(den) * log2(e);  o = O^T * exp(-ln(den))  (normalize on evac)
Host transposes O^T back and assembles the full outputs.
"""

import functools
import math
import os
import sys

import ml_dtypes
import numpy as np

for _p in ("/opt/trn_rl_repo", "/root/.axon_site/_ro/trn_rl_repo"):
    if _p not in sys.path:
        sys.path.insert(0, _p)

import concourse.bacc as bacc
import concourse.mybir as mybir
import concourse.tile as tile
from concourse.bass_utils import run_bass_kernel_spmd

F32 = mybir.dt.float32
F32R = mybir.dt.float32r
BF16 = mybir.dt.bfloat16

B, S, H, DN, DR, DV = 2, 2048, 16, 128, 64, 128
NCORES = 8
NPAIRS = (B * H) // NCORES  # 4 (b,h) pairs per core
MC = 512  # m-chunk width = one fp32 PSUM bank
SM_SCALE = (DN + DR) ** -0.5
LOG2E = 1.0 / math.log(2.0)

# ---- yarn rope tables (constants; mirrors reference.py) --------------------
BASE = 10000.0
SCALING_FACTOR = 1.0
ORIG_MAX_POS = 4096
BETA_FAST, BETA_SLOW = 32, 1
MSCALE = 0.707
MSCALE_ALL_DIM = 0.707


def _yarn_cos_sin(seq_len: int, dim: int):
    def corr_dim(num_rot):
        return (
            dim
            * math.log(ORIG_MAX_POS / (num_rot * 2 * math.pi))
            / (2 * math.log(BASE))
        )

    low = max(math.floor(corr_dim(BETA_FAST)), 0)
    high = min(math.ceil(corr_dim(BETA_SLOW)), dim - 1)
    hi = high + 0.001 if low == high else high
    ramp = np.clip((np.arange(dim // 2, dtype=np.float32) - low) / (hi - low), 0.0, 1.0)
    inv_freq_mask = 1.0 - ramp
    freq_extra = 1.0 / BASE ** (np.arange(0, dim, 2, dtype=np.float32) / dim)
    freq_inter = freq_extra / SCALING_FACTOR
    inv_freq = freq_inter * (1.0 - inv_freq_mask) + freq_extra * inv_freq_mask
    t = np.arange(seq_len, dtype=np.float32)
    freqs = np.outer(t, inv_freq)

    def get_mscale(s, m):
        return 1.0 if s <= 1 else 0.1 * m * math.log(s) + 1.0

    _m = get_mscale(SCALING_FACTOR, MSCALE) / get_mscale(SCALING_FACTOR, MSCALE_ALL_DIM)
    emb = np.concatenate([freqs, freqs], axis=-1)  # [S, dim]
    return (np.cos(emb) * _m).astype(np.float32), (np.sin(emb) * _m).astype(np.float32)


# ---- device kernel builder -------------------------------------------------
def build_nc(npairs=NPAIRS, s=S, mm_dt=F32R, debug=False):
    """Build the single-core SPMD Bass program (same program on all cores)."""
    ntt = s // 128  # number of 128-wide n (kv) tiles
    nmc = s // MC  # number of 512-wide m (query) chunks
    ngroups = (npairs + 1) // 2
    mdt = mm_dt  # dtype of every matmul operand, typed end-to-end for walrus
    nc = bacc.Bacc("TRN2", target_bir_lowering=False, debug=debug)

    q_nope_t = nc.dram_tensor("q_nope_t", [npairs, 128, s], mdt, kind="ExternalInput")
    k_nope_t = nc.dram_tensor("k_nope_t", [npairs, 128, s], mdt, kind="ExternalInput")
    q_pe_nat = nc.dram_tensor("q_pe_nat", [ngroups, 128, s], mdt, kind="ExternalInput")
    q_pe_rot = nc.dram_tensor("q_pe_rot", [ngroups, 128, s], mdt, kind="ExternalInput")
    k_pe_nat = nc.dram_tensor("k_pe_nat", [64, s], mdt, kind="ExternalInput")
    k_pe_rot = nc.dram_tensor("k_pe_rot", [64, s], mdt, kind="ExternalInput")
    cs = nc.dram_tensor("cs", [2, 128, s], mdt, kind="ExternalInput")
    v_t = nc.dram_tensor("v_t", [npairs, 128, ntt, 128], mdt, kind="ExternalInput")
    o_t = nc.dram_tensor("o_t", [npairs, 128, s], F32, kind="ExternalOutput")
    lse = nc.dram_tensor("lse", [npairs, s], F32, kind="ExternalOutput")

    Exp = mybir.ActivationFunctionType.Exp
    Ln = mybir.ActivationFunctionType.Ln
    Copy = mybir.ActivationFunctionType.Copy

    with tile.TileContext(nc) as tc:
        with (
            tc.tile_pool(name="const", bufs=1) as constp,
            tc.tile_pool(name="ropein", bufs=1) as ropein,
            tc.tile_pool(name="ropeout", bufs=1) as ropeout,
            tc.tile_pool(name="pairin", bufs=2) as pairin,
            tc.tile_pool(name="pmat", bufs=4) as pmat,
            tc.tile_pool(name="oun", bufs=2) as ounp,
            tc.tile_pool(name="rbp", bufs=1) as rbp,
            tc.tile_pool(name="rows", bufs=1) as rows,
            tc.tile_pool(name="ps_s", bufs=2, space="PSUM") as ps_s,
            tc.tile_pool(name="ps_o", bufs=2, space="PSUM") as ps_o,
            tc.tile_pool(name="ps_d", bufs=2, space="PSUM") as ps_d,
        ):
            cos2 = constp.tile([128, s], mdt)
            nc.scalar.dma_start(cos2, cs[0])
            sin2 = constp.tile([128, s], mdt)
            nc.scalar.dma_start(sin2, cs[1])
            ones = constp.tile([128, 128], mdt)
            if mdt == F32:
                nc.vector.memset(ones, 1.0)
            else:
                ones_f32 = constp.tile([128, 128], F32)
                nc.vector.memset(ones_f32, 1.0)
                nc.vector.tensor_copy(ones, ones_f32)  # rounds to mm dtype

            # k_pe rope - shared by every pair on this core (same b)
            k_nat = ropein.tile([64, s], mdt, tag="knat")
            nc.scalar.dma_start(k_nat, k_pe_nat[:])
            k_rot = ropein.tile([64, s], mdt, tag="krot")
            nc.scalar.dma_start(k_rot, k_pe_rot[:])
            kp = ropeout.tile([64, s], mdt, tag="kp")
            nc.vector.tensor_mul(k_rot, k_rot, sin2[0:64, :])  # in-place
            nc.vector.tensor_mul(kp, k_nat, cos2[0:64, :])
            nc.vector.tensor_add(kp, kp, k_rot)

            # q_pe rope - two pairs packed per [128, s] group
            qps = []
            for g in range(ngroups):
                qn = ropein.tile([128, s], mdt, tag="qnat")
                nc.gpsimd.dma_start(qn, q_pe_nat[g])
                qr = ropein.tile([128, s], mdt, tag="qrot")
                nc.gpsimd.dma_start(qr, q_pe_rot[g])
                qp = ropeout.tile([128, s], mdt, tag=f"qp{g}")
                nc.vector.tensor_mul(qr, qr, sin2)  # in-place
                nc.vector.tensor_mul(qp, qn, cos2)
                nc.vector.tensor_add(qp, qp, qr)
                # matmul requires lhsT/rhs at the same base partition; move the
                # packed upper half (odd pair) down to base 0 via SBUF-SBUF DMA
                qp_hi = ropeout.tile([64, s], mdt, tag=f"qp_hi{g}")
                nc.gpsimd.dma_start(qp_hi, qp[64:128, :])
                qps.append((qp, qp_hi))

            NTB = MC // 128  # n-tiles per m-chunk (4)
            for p in range(npairs):
                g, half = divmod(p, 2)
                qp_pair = qps[g][0][0:64, :] if half == 0 else qps[g][1][:, :]
                qn_t = pairin.tile([128, s], mdt, tag="qn")
                nc.sync.dma_start(qn_t, q_nope_t[p])
                kn_t = pairin.tile([128, s], mdt, tag="kn")
                nc.sync.dma_start(kn_t, k_nope_t[p])
                v_sb = pairin.tile([128, ntt, 128], mdt, tag="v")
                nc.sync.dma_start(v_sb, v_t[p])
                o_un = ounp.tile([128, s], F32, tag="oun")
                den_sb = rows.tile([1, s], F32, tag="den")

                for mc in range(nmc):
                    msl = slice(mc * MC, (mc + 1) * MC)
                    nlim = (mc + 1) * NTB
                    psum_o = ps_o.tile([128, MC], F32)
                    psum_d = ps_d.tile([128, MC], F32)
                    pending = None  # software-pipeline PV/den one step behind

                    def emit_pv(p_t, pvs, nts, nlim=nlim, psum_o=psum_o, psum_d=psum_d):
                        for h, (vs, nt) in enumerate(zip(pvs, nts)):
                            hsl = slice(h * 512 + vs, (h + 1) * 512)
                            nc.tensor.matmul(
                                psum_o[:, vs:],
                                v_sb[:, nt, :],
                                p_t[:, hsl],
                                start=(nt == 0),
                                stop=(nt == nlim - 1),
                            )
                            nc.tensor.matmul(
                                psum_d[:, vs:],
                                ones,
                                p_t[:, hsl],
                                start=(nt == 0),
                                stop=(nt == nlim - 1),
                            )

                    for t in range(nlim // 2):
                        nts = (2 * t, 2 * t + 1)
                        psum_s = ps_s.tile([128, 2, MC], F32)
                        p_t = pmat.tile([128, 2, MC], mdt, tag="pt")
                        offs, pvs = [], []
                        for h, nt in enumerate(nts):
                            diag_k = nt - (nlim - NTB)  # >=0 inside diag band
                            off = 128 * diag_k if diag_k > 0 else 0
                            vs = min(off, 256)  # keep f32r moving width >= 256
                            offs.append(off)
                            pvs.append(vs)
                            nsl = slice(nt * 128, (nt + 1) * 128)
                            qsl = slice(mc * MC + vs, (mc + 1) * MC)
                            nc.tensor.matmul(
                                psum_s[:, h, vs:],
                                kn_t[:, nsl],
                                qn_t[:, qsl],
                                start=True,
                                stop=False,
                            )
                            nc.tensor.matmul(
                                psum_s[:, h, vs:],
                                kp[:, nsl],
                                qp_pair[:, qsl],
                                start=False,
                                stop=True,
                            )
                        # one wide exp over both halves when fully valid;
                        # split per half when a diagonal offset leaves the
                        # head of a half unwritten in PSUM
                        if pvs[0] == 0 and pvs[1] == 0:
                            nc.scalar.activation(
                                p_t.rearrange("p a b -> p (a b)"),
                                psum_s.rearrange("p a b -> p (a b)"),
                                Exp,
                                scale=SM_SCALE,
                            )
                        else:
                            for h in range(2):
                                nc.scalar.activation(
                                    p_t[:, h, pvs[h] :],
                                    psum_s[:, h, pvs[h] :],
                                    Exp,
                                    scale=SM_SCALE,
                                )
                        for h, nt in enumerate(nts):
                            diag_k = nt - (nlim - NTB)
                            if diag_k >= 0:
                                off, vs = offs[h], pvs[h]
                                w = off + 128 - vs
                                # keep cols where m_local >= n_local:
                                # iota = (y + vs - off) - x >= 0
                                nc.gpsimd.affine_select(
                                    out=p_t[:, h, vs : off + 128],
                                    in_=p_t[:, h, vs : off + 128],
                                    compare_op=mybir.AluOpType.is_ge,
                                    fill=0.0,
                                    base=vs - off,
                                    pattern=[[1, w]],
                                    channel_multiplier=-1,
                                )
                        if pending is not None:
                            emit_pv(*pending)
                        pending = (p_t.rearrange("p a b -> p (a b)"), pvs, nts)
                    emit_pv(*pending)

                    # evacuate den row (all psum_d rows are identical; read the
                    # one matching this chunk's den_sb partition to stay aligned)
                    nc.vector.tensor_copy(
                        den_sb[:, msl], psum_d[0:1, :]
                    )
                    # evacuate unnormalized O^T
                    nc.vector.tensor_copy(o_un[:, msl], psum_o)

                # deferred per-pair epilogue: one ACT table context
                nc.scalar.activation(den_sb, den_sb, Ln)
                lse_sb = rows.tile([1, s], F32, tag="lse")
                nc.scalar.activation(lse_sb, den_sb, Copy, scale=LOG2E)
                nc.scalar.activation(den_sb, den_sb, Exp, scale=-1.0)  # 1/den
                rb = rbp.tile([128, s], F32, tag="rb")
                nc.gpsimd.partition_broadcast(rb, den_sb)
                nc.vector.tensor_mul(o_un, o_un, rb)
                nc.sync.dma_start(o_t[p], o_un)
                nc.sync.dma_start(lse[p : p + 1, :], lse_sb)

    nc.compile()
    return nc


# ---- host-side input/output marshalling ------------------------------------
def _pair_index(core, i):
    idx = core * NPAIRS + i  # b-major: cores 0-3 -> b=0, cores 4-7 -> b=1
    return idx // H, idx % H


def make_in_maps(q_nope, q_pe, k_nope, k_pe, v, npairs=NPAIRS, s=S, ncores=NCORES):
    """Slice + lay out the full inputs into per-core input dicts."""
    cos, sin = _yarn_cos_sin(s, DR)
    # device computes qp[r] = nat[r]*cos[r] + nat[(r+32)%64]*sin_eff[r]
    sin_eff = sin.copy()
    sin_eff[:, : DR // 2] *= -1.0
    cosT = np.ascontiguousarray(cos.T)  # [64, S]
    sinT = np.ascontiguousarray(sin_eff.T)
    cs = np.stack(
        [np.concatenate([cosT, cosT], 0), np.concatenate([sinT, sinT], 0)]
    )  # [2, 128, S]

    ntt = s // 128
    ngroups = (npairs + 1) // 2
    in_maps = []
    for c in range(ncores):
        pairs = [_pair_index(c, i) for i in range(npairs)]
        b0 = pairs[0][0]
        assert all(b == b0 for b, _ in pairs), "core must own a single batch"
        qnt = np.stack([np.ascontiguousarray(q_nope[b, :, h, :].T) for b, h in pairs])
        knt = np.stack([np.ascontiguousarray(k_nope[b, :, h, :].T) for b, h in pairs])
        vt = np.stack(
            [
                np.ascontiguousarray(
                    v[b, :, h, :].reshape(ntt, 128, DV).transpose(1, 0, 2)
                )
                for b, h in pairs
            ]
        )
        q_nat_groups, q_rot_groups = [], []
        for gi in range(ngroups):
            mats_nat, mats_rot = [], []
            for i in (2 * gi, min(2 * gi + 1, npairs - 1)):
                b, h = pairs[i]
                nat = np.ascontiguousarray(q_pe[b, :, h, :].T)  # [64, S]
                mats_nat.append(nat)
                mats_rot.append(np.concatenate([nat[DR // 2 :], nat[: DR // 2]], 0))
            q_nat_groups.append(np.concatenate(mats_nat, 0))
            q_rot_groups.append(np.concatenate(mats_rot, 0))
        k_nat = np.ascontiguousarray(k_pe[b0, :, 0, :].T)  # [64, S]
        k_rot = np.concatenate([k_nat[DR // 2 :], k_nat[: DR // 2]], 0)
        dt = MM_NP_DT
        in_maps.append(
            {
                "q_nope_t": qnt.astype(dt),
                "k_nope_t": knt.astype(dt),
                "q_pe_nat": np.stack(q_nat_groups).astype(dt),
                "q_pe_rot": np.stack(q_rot_groups).astype(dt),
                "k_pe_nat": k_nat.astype(dt),
                "k_pe_rot": k_rot.astype(dt),
                "cs": cs.astype(dt),
                "v_t": vt.astype(dt),
            }
        )
    return in_maps


def assemble_outputs(results, npairs=NPAIRS, s=S, ncores=NCORES):
    o = np.empty((B, s, H, DV), np.float32)
    lse_full = np.empty((B, s, H), np.float32)
    for c in range(ncores):
        r = results[c]
        for i in range(npairs):
            b, h = _pair_index(c, i)
            o[b, :, h, :] = r["o_t"][i].T
            lse_full[b, :, h] = r["lse"][i]
    return o.reshape(B * s, H, DV), lse_full


MM_DT_NAME = os.environ.get("MLA_MM_DT", "f32r")
MM_DT = {"f32r": F32R, "bf16": BF16, "f32": F32}[MM_DT_NAME]
MM_NP_DT = {"f32r": np.float32, "bf16": ml_dtypes.bfloat16, "f32": np.float32}[
    MM_DT_NAME
]


@functools.lru_cache(maxsize=1)
def _get_nc():
    return build_nc(mm_dt=MM_DT)


def kernel(q_nope, q_pe, k_nope, k_pe, v, _trace=False):
    q_nope = np.asarray(q_nope, np.float32)
    q_pe = np.asarray(q_pe, np.float32)
    k_nope = np.asarray(k_nope, np.float32)
    k_pe = np.asarray(k_pe, np.float32)
    v = np.asarray(v, np.float32)
    in_maps = make_in_maps(q_nope, q_pe, k_nope, k_pe, v)
    nc = _get_nc()
    res = run_bass_kernel_spmd(nc, in_maps, core_ids=list(range(NCORES)), trace=_trace)
    out = assemble_outputs(res.results)
    if _trace:
        return out, res
    return out
